# revision 26
# baseline (speedup 1.0000x reference)
"""Trainium2 Bass kernel for bipartite cross-batch attention.

Reference computation (per full inputs):
  q  = LN(qx; gq,bq) @ Wq.T            -> [Bq, H, hd]
  k  = LN(kx; gk,bk) @ Wk.T            -> [Bk, Nk, H, hd]
  a  = softmax(q.k * hd^-0.5, axis=Nk) -> [Bq, Bk, H, Nk]
  w  = a.sum(H)                        -> [Bq, Bk, Nk]
  out= einsum('knc,qkn->qkc', kx, w)   -> [Bq, Bk, C]

Bq=128, Bk=128, Nk=256, C=1024, H=16, hd=64.

Distribution: shard Bk across the 8 cores (16 k-batches each); the softmax
axis Nk is fully core-local so there are no collectives.

Host-side prep: gq/gk fold into the projection weights; bk drops
(softmax-invariant); bq folds into a per-output-channel bias; hd^-0.5 and
the fp8 weight prescale (x32, exact power of two) fold into qT. The K-side
LayerNorm runs on the host while building the transposed copies of kx.

Precision: the K projection contracts C=1024 channels; the last NFP8
channels run as double-pumped fp8(e4m3) DoubleRow matmuls, the rest bf16.
Both weight halves are prescaled by 32 so fp8 weight magnitudes clear the
e4m3 subnormal range; qT carries the 1/32 so scores are exact.

Per-pair engine assignment (from measured op costs):
  - PE: projections, scores, 9 heads of the softmax head-sum as
    accumulating diag(1/d_h) matmuls into PSUM, w transposes, AV.
  - Act: kproj PSUM drains, exps, hsum drain, AV output drains.
  - DVE: softmax denominators (tensor_reduce), reciprocal, diag tile
    builds, 7 heads of the head-sum as scalar_tensor_tensor, merge add,
    wT copies.
  - Sync: all steady-state DMAs. GpSimd: startup DMAs only (its SBUF
    port is shared with the DVE, so giving it elementwise work slows
    the DVE down).
Pair p's den/chain run during p+1's projection stream; transposes/AV/
store land early in p+2.
"""

import numpy as np
import ml_dtypes

BF16 = ml_dtypes.bfloat16
E4M3 = ml_dtypes.float8_e4m3
H, C, HD = 16, 1024, 64
BQ, BK, NK = 128, 128, 256
NCORES = 8
BKL = BK // NCORES  # k-batches per core
PAIRS = BKL // 2
EPS = 1e-5

NFP8 = 512            # contract channels in fp8 (0, 256 or 512)
NBF = C - NFP8
NCHB = NBF // 128     # bf16 contract chunks
NCH8 = NFP8 // 256    # fp8 DoubleRow contract chunks
WSC = 32.0            # weight prescale (power of two; 1/WSC folded into qT)
HPE = 12              # heads summed on PE (diag matmuls); rest on DVE
DEN_A = 10            # heads of pair p's den reduced already in window p

_CACHE: dict = {}


def _build():
    from contextlib import ExitStack
    from concourse import bacc, tile, mybir

    f32 = mybir.dt.float32
    bf16 = mybir.dt.bfloat16
    f8e4 = mybir.dt.float8e4
    Alu = mybir.AluOpType
    Act = mybir.ActivationFunctionType
    DR = mybir.MatmulPerfMode.DoubleRow

    nc = bacc.Bacc("TRN2", target_bir_lowering=False, debug=False)

    kxtb_d = nc.dram_tensor(
        "kxtb", [PAIRS, 128, NCHB, 2 * NK], bf16, kind="ExternalInput").ap()
    if NCH8:
        kxt8_d = nc.dram_tensor(
            "kxt8", [PAIRS, 128, NCH8, 2, 2 * NK], f8e4, kind="ExternalInput").ap()
    kxn_d = nc.dram_tensor("kxn", [BKL, 128, 2, C], bf16, kind="ExternalInput").ap()
    qt_d = nc.dram_tensor("qt", [128, 8, 128], bf16, kind="ExternalInput").ap()
    wkb_d = nc.dram_tensor("wkb", [128, 8, NCHB, 128], bf16, kind="ExternalInput").ap()
    if NCH8:
        wk8_d = nc.dram_tensor(
            "wk8", [128, 8, NCH8, 2, 128], f8e4, kind="ExternalInput").ap()
    id_d = nc.dram_tensor("ident", [128, 128], bf16, kind="ExternalInput").ap()
    out_d = nc.dram_tensor("out", [BKL, BQ, C], bf16, kind="ExternalOutput").ap()

    with tile.TileContext(nc) as tc, ExitStack() as ctx:
        const = ctx.enter_context(tc.tile_pool(name="const", bufs=1))
        ktb_p = ctx.enter_context(tc.tile_pool(name="ktb", bufs=2))
        kt8_p = ctx.enter_context(tc.tile_pool(name="kt8", bufs=2))
        kn_p = ctx.enter_context(tc.tile_pool(name="kn", bufs=8))
        kj_p = ctx.enter_context(tc.tile_pool(name="kj", bufs=2))
        ex_p = ctx.enter_context(tc.tile_pool(name="ex", bufs=3))
        den_p = ctx.enter_context(tc.tile_pool(name="den", bufs=4))
        dg_p = ctx.enter_context(tc.tile_pool(name="dg", bufs=2))
        w_p = ctx.enter_context(tc.tile_pool(name="w", bufs=4))
        os_p = ctx.enter_context(tc.tile_pool(name="os", bufs=4))
        # PSUM: 8 banks: kp 2 + sc 3 + hs 1 + tp 1 + av 1
        pp_kp = ctx.enter_context(tc.tile_pool(name="pp_kp", bufs=2, space="PSUM"))
        pp_sc = ctx.enter_context(tc.tile_pool(name="pp_sc", bufs=3, space="PSUM"))
        pp_hs = ctx.enter_context(tc.tile_pool(name="pp_hs", bufs=1, space="PSUM"))
        pp_avt = ctx.enter_context(tc.tile_pool(name="pp_avt", bufs=1, space="PSUM"))

        # ---- constants ----
        wkb_t = const.tile([128, 8, NCHB, 128], bf16)
        if NCH8:
            wk8_t = const.tile([128, 8, NCH8, 2, 128], f8e4)
        id_t = const.tile([128, 128], bf16)
        qT = const.tile([128, 8, 128], bf16)

        ktb_tiles, kt8_tiles, kn_tiles = {}, {}, {}

        def emit_kt(bp, split=False):
            ktb = ktb_p.tile([128, NCHB, 2 * NK], bf16, tag="ktb")
            if split:
                for i in range(NCHB):
                    nc.sync.dma_start(ktb[:, i, :], kxtb_d[bp, :, i, :])
            else:
                nc.sync.dma_start(ktb[:], kxtb_d[bp])
            ktb_tiles[bp] = ktb
            if NCH8:
                kt8 = kt8_p.tile([128, NCH8, 2, 2 * NK], f8e4, tag="kt8")
                if split:
                    for d in range(NCH8):
                        nc.sync.dma_start(kt8[:, d, :, :], kxt8_d[bp, :, d, :, :])
                else:
                    nc.sync.dma_start(kt8[:], kxt8_d[bp])
                kt8_tiles[bp] = kt8

        def emit_kn(b, eng=None):
            kn_t = kn_p.tile([128, 2, C], bf16, tag="kn")
            (eng or nc.sync).dma_start(kn_t[:], kxn_d[b])
            kn_tiles[b] = kn_t

        # ---- startup DMAs (ordered for earliest proj(0,0) start) ----
        for i in range(NCHB):
            nc.scalar.dma_start(wkb_t[:, 0, i], wkb_d[:, 0, i])
        if NCH8:
            nc.scalar.dma_start(wk8_t[:, 0:2], wk8_d[:, 0:2])
        emit_kt(0, split=True)
        nc.scalar.dma_start(wkb_t[:, 1], wkb_d[:, 1])
        for j in range(2, 8):
            nc.gpsimd.dma_start(wkb_t[:, j], wkb_d[:, j])
        if NCH8:
            nc.scalar.dma_start(wk8_t[:, 2:8], wk8_d[:, 2:8])
        nc.gpsimd.dma_start(qT[:], qt_d[:])
        nc.gpsimd.dma_start(id_t[:], id_d[:])
        emit_kn(0, nc.gpsimd)
        emit_kn(1, nc.gpsimd)

        ex_tiles, wps_tiles, wsb_tiles, wacc_tiles = {}, {}, {}, {}
        w_tiles, iden_tiles, diag_tiles = {}, {}, {}

        den_tiles = {}

        def emit_den_a(bp):
            # DVE: dens for heads [0, DEN_A) of pair bp, in window bp
            extile = ex_tiles[bp]
            dens = den_p.tile([128, 32], f32, tag="dens")
            den_tiles[bp] = dens
            nc.vector.tensor_reduce(
                dens[:, 0:2 * DEN_A], extile[:, 0:DEN_A],
                mybir.AxisListType.X, Alu.add)

        def emit_den_b(bp):
            # DVE: remaining dens + reciprocal, in window bp+1
            extile = ex_tiles[bp]
            dens = den_tiles.pop(bp)
            nc.vector.tensor_reduce(
                dens[:, 2 * DEN_A:32], extile[:, DEN_A:16],
                mybir.AxisListType.X, Alu.add)
            idens = den_p.tile([128, 32], f32, tag="idens")
            nc.vector.reciprocal(idens[:], dens[:])
            iden_tiles[bp] = idens

        def emit_diag(bp, hs):
            # DVE: diag_h = ident * (1/d_h) as [128, 2, 128] per (h, t)
            idens = iden_tiles[bp]
            diag = diag_tiles.get(bp)
            if diag is None:
                diag = dg_p.tile([128, 16, 2, 128], bf16, tag="diag")
                diag_tiles[bp] = diag
            for h in hs:
                for t in range(2):
                    nc.vector.tensor_scalar(
                        diag[:, h, t, :], id_t[:],
                        idens[:, 2 * h + t:2 * h + t + 1], None, op0=Alu.mult)

        def emit_hsum_pe(bp, hs, start=False, stop=False):
            # PE: wps[q, t, n] += r_h[q] * e_h[q, t, n], accumulated in PSUM
            extile, diag = ex_tiles[bp], diag_tiles[bp]
            wps = wps_tiles.get(bp)
            if wps is None:
                wps = pp_hs.tile([128, 2, NK], f32, tag="hs")
                wps_tiles[bp] = wps
            for idx, h in enumerate(hs):
                for t in range(2):
                    nc.tensor.matmul(
                        wps[:, t, :], diag[:, h, t, :], extile[:, h, t, :],
                        start=(start and idx == 0 and t == 0),
                        stop=(stop and idx == len(hs) - 1 and t == 1),
                    )

        def emit_diag_act(bp, hs):
            # Act builds diag tiles (tail only; Act is idle there)
            idens = iden_tiles[bp]
            diag = diag_tiles.get(bp)
            if diag is None:
                diag = dg_p.tile([128, 16, 2, 128], bf16, tag="diag")
                diag_tiles[bp] = diag
            for h in hs:
                for t in range(2):
                    nc.scalar.activation(
                        diag[:, h, t, :], id_t[:], Act.Copy,
                        scale=idens[:, 2 * h + t:2 * h + t + 1])

        def emit_hsum_drain(bp):
            # tail only: all heads summed on PE; drain psum -> w tile
            wsb = w_p.tile([128, 2, NK], bf16, tag="wacc")
            nc.scalar.copy(wsb[:], wps_tiles.pop(bp)[:])
            diag_tiles.pop(bp)
            ex_tiles.pop(bp)
            iden_tiles.pop(bp)
            w_tiles[bp] = ("w", wsb)

        def emit_stt(bp, hs, seed=False):
            # DVE: wacc += e_h * (1/d_h); the first op seeds from the PE
            # half's PSUM accumulator directly (kills merge + drain)
            extile, idens = ex_tiles[bp], iden_tiles[bp]
            wacc = wacc_tiles.get(bp)
            if wacc is None:
                wacc = w_p.tile([128, 2, NK], bf16, tag="wacc")
                wacc_tiles[bp] = wacc
            for idx, h in enumerate(hs):
                for t in range(2):
                    in1 = (wps_tiles[bp][:, t, :] if (seed and idx == 0)
                           else wacc[:, t, :])
                    nc.vector.scalar_tensor_tensor(
                        wacc[:, t, :], extile[:, h, t, :],
                        idens[:, 2 * h + t:2 * h + t + 1], in1,
                        op0=Alu.mult, op1=Alu.add)

        def emit_finish(bp):
            # bookkeeping: wacc is the final w
            ex_tiles.pop(bp)
            iden_tiles.pop(bp)
            diag_tiles.pop(bp)
            wps_tiles.pop(bp)
            w_tiles[bp] = ("w", wacc_tiles.pop(bp))

        def emit_transpose_w(bp):
            # PE: 4 transposes into one psum bank; DVE: one [128,512] copy
            _, wfin = w_tiles.pop(bp)
            wT = w_p.tile([128, 2, 2, 128], bf16, tag="wT")
            wtp = pp_avt.tile([128, 512], bf16, tag="tp", bufs=1)
            for t in range(2):
                for u in range(2):
                    k = 2 * t + u
                    nc.tensor.transpose(
                        wtp[:, k * 128:(k + 1) * 128],
                        wfin[:, t, u * 128:(u + 1) * 128], id_t[:])
            nc.vector.tensor_copy(
                wT[:].rearrange("p t u q -> p (t u q)"), wtp[:])
            w_tiles[bp] = ("wT", wT)

        def emit_store_t(bp, t):
            # AV matmuls + Act drains + store for one batch of pair bp
            _, wT = w_tiles[bp]
            if t == 1:
                w_tiles.pop(bp)
            b = 2 * bp + t
            kn_t = kn_tiles.pop(b)
            out_sb = os_p.tile([BQ, C], bf16, tag="osb")
            for m in range(2):
                # m=1 borrows the transpose bank (free after j==1) so the
                # m=1 matmuls never wait on m=0's Act drain
                tag = "av" if m == 0 else "tp"
                avp = pp_avt.tile([BQ, 512], f32, tag=tag, bufs=1)
                for u in range(2):
                    nc.tensor.matmul(
                        avp[:], wT[:, t, u, :],
                        kn_t[:, u, m * 512:(m + 1) * 512],
                        start=(u == 0), stop=(u == 1),
                    )
                nc.scalar.copy(out_sb[:, m * 512:(m + 1) * 512], avp[:])
                nc.sync.dma_start(
                    out_d[b, :, m * 512:(m + 1) * 512],
                    out_sb[:, m * 512:(m + 1) * 512])

        HS_PE = list(range(HPE))           # heads on PE
        HS_DVE = list(range(HPE, 16))      # heads on DVE

        # ---- main paired loop ----
        for bp in range(PAIRS):
            ktb = ktb_tiles.pop(bp)
            kt8 = kt8_tiles.pop(bp) if NCH8 else None
            kjp = kj_p.tile([128, 8, 2 * NK], bf16, tag="kj")
            extile = ex_p.tile([128, 16, 2, NK], bf16, tag="ex")
            ex_tiles[bp] = extile

            def emit_kproj(j, ktb=ktb, kt8=kt8, kjp=kjp):
                kpp = pp_kp.tile([BQ, 2 * NK], f32, tag="kp")
                for i in range(NCHB):
                    nc.tensor.matmul(
                        kpp[:], wkb_t[:, j, i, :], ktb[:, i, :],
                        start=(i == 0), stop=(NCH8 == 0 and i == NCHB - 1),
                    )
                for d in range(NCH8):
                    nc.tensor.matmul(
                        kpp[:], wk8_t[:, j, d, :, :], kt8[:, d, :, :],
                        start=False, stop=(d == NCH8 - 1),
                        perf_mode=DR,
                    )
                nc.scalar.copy(kjp[:, j, :], kpp[:])

            def emit_score(h, kjp=kjp, extile=extile):
                j, off = h // 2, (h % 2) * 64
                scp = pp_sc.tile([BQ, 2 * NK], f32, tag="sc")
                nc.tensor.matmul(
                    scp[:], qT[off:off + 64, j, :], kjp[off:off + 64, j, :],
                    start=True, stop=True,
                )
                nc.scalar.activation(extile[:, h, :, :], scp[:], Act.Exp)

            for j in range(8):
                emit_kproj(j)
                if j == 0:
                    if bp + 1 < PAIRS:
                        emit_kt(bp + 1)
                        emit_kn(2 * bp + 2)
                        emit_kn(2 * bp + 3)
                    if bp >= 1:
                        emit_den_b(bp - 1)
                if j == 1:
                    if bp >= 2:
                        emit_transpose_w(bp - 2)
                    if bp >= 1:
                        emit_diag(bp - 1, HS_PE[0:4])
                if j == 2:
                    if bp >= 2:
                        emit_store_t(bp - 2, 0)
                    if bp >= 1:
                        emit_diag(bp - 1, HS_PE[4:8])
                        emit_hsum_pe(bp - 1, HS_PE[0:4], start=True)
                if j == 3 and bp >= 1:
                    emit_diag(bp - 1, HS_PE[8:HPE])
                    emit_hsum_pe(bp - 1, HS_PE[4:8])
                if j == 4:
                    if bp >= 2:
                        emit_store_t(bp - 2, 1)
                    if bp >= 1:
                        emit_hsum_pe(bp - 1, HS_PE[8:HPE], stop=True)
                if j == 5 and bp >= 1:
                    emit_stt(bp - 1, HS_DVE[0:2], seed=True)
                if j == 6:
                    if bp >= 1:
                        emit_stt(bp - 1, HS_DVE[2:4])
                    emit_den_a(bp)
                if j == 7 and bp >= 1:
                    emit_finish(bp - 1)
                if j >= 1:
                    emit_score(2 * (j - 1))
                    emit_score(2 * (j - 1) + 1)
            for h in range(14, 16):
                emit_score(h)

        # ---- epilogue: flush pairs 6 and 7 ----
        bp = PAIRS - 1
        emit_transpose_w(bp - 1)
        emit_store_t(bp - 1, 0)
        emit_den_b(bp)
        emit_store_t(bp - 1, 1)
        # tail pair: all heads on PE; diag builds split DVE/Act
        # (Act copies cost ~480ns vs DVE ~254ns, so DVE takes more)
        emit_diag(bp, range(0, 11))
        emit_diag_act(bp, range(11, 16))
        emit_hsum_pe(bp, range(0, 11), start=True)
        emit_hsum_pe(bp, range(11, 16), stop=True)
        emit_hsum_drain(bp)
        emit_transpose_w(bp)
        emit_store_t(bp, 0)
        emit_store_t(bp, 1)

    nc.compile()
    return nc


def _prep(qx, kx, gq, bq, gk, bk, Wq, Wk):
    scale = HD ** -0.5
    qx_h = np.ascontiguousarray(qx[:, 0, :], dtype=np.float32)
    Wqp = (Wq * gq[None, :]).T.astype(np.float32) * scale  # [c, o]
    Wkp = (Wk * gk[None, :]).T.astype(np.float32)  # [c, o]
    # weight prescale x32 (power of 2): fp8 half clears e4m3 subnormals;
    # bf16 half matches so the PSUM accumulation is uniformly 32x; the
    # 1/32 folds into qT (scores exact).
    Wks = Wkp * WSC
    wkb_h = np.ascontiguousarray(
        Wks[:NBF].reshape(NCHB, 128, 8, 128).transpose(1, 2, 0, 3)
    ).astype(BF16)  # [p, j, i, o]
    if NCH8:
        wk8_h = np.ascontiguousarray(
            Wks[NBF:].reshape(NCH8, 128, 2, 8, 128).transpose(1, 3, 0, 2, 4)
        ).astype(E4M3)  # [p, j, d, s, o]

    qm = qx_h.mean(-1, keepdims=True)
    qv = qx_h.var(-1, keepdims=True)
    lnq = ((qx_h - qm) / np.sqrt(qv + EPS)).astype(BF16).astype(np.float32)
    q = lnq @ Wqp.astype(BF16).astype(np.float32)
    q += scale * (bq[None, :] @ Wq.T)
    qt_h = np.ascontiguousarray(
        (q / WSC).T.reshape(8, 128, 128).transpose(1, 0, 2)).astype(BF16)
    id_h = np.eye(128, dtype=np.float32).astype(BF16)

    shared = dict(qt=qt_h, wkb=wkb_h, ident=id_h)
    if NCH8:
        shared["wk8"] = wk8_h
    in_maps = []
    for i in range(NCORES):
        kxl = np.asarray(kx[i * BKL:(i + 1) * BKL], dtype=np.float32)
        m = kxl.mean(axis=-1, keepdims=True)
        v = kxl.var(axis=-1, keepdims=True)
        kln = (kxl - m) / np.sqrt(v + EPS)  # [b, n, c]
        klt = (
            kln.reshape(PAIRS, 2, NK, C)
            .transpose(0, 3, 1, 2)  # [bp, c, t, n]
            .reshape(PAIRS, C, 2 * NK)
        )
        kxtb_h = np.ascontiguousarray(
            klt[:, :NBF].reshape(PAIRS, NCHB, 128, 2 * NK).transpose(0, 2, 1, 3)
        ).astype(BF16)  # [bp, p, i, tn]
        d = dict(kxtb=kxtb_h, **shared)
        if NCH8:
            d["kxt8"] = np.ascontiguousarray(
                klt[:, NBF:].reshape(PAIRS, NCH8, 128, 2, 2 * NK)
                .transpose(0, 2, 1, 3, 4)
            ).astype(E4M3)  # [bp, p, d, s, tn]
        d["kxn"] = np.ascontiguousarray(
            kxl.reshape(BKL, 2, 128, C).transpose(0, 2, 1, 3)
        ).astype(BF16)
        in_maps.append(d)
    return in_maps


def kernel(qx, kx, gq, bq, gk, bk, Wq, Wk):
    from concourse.bass_utils import run_bass_kernel_spmd

    qx, kx, gq, bq, gk, bk, Wq, Wk = (
        np.asarray(a, dtype=np.float32)
        for a in (qx, kx, gq, bq, gk, bk, Wq, Wk)
    )
    if "nc" not in _CACHE:
        _CACHE["nc"] = _build()
    nc = _CACHE["nc"]
    in_maps = _prep(qx, kx, gq, bq, gk, bk, Wq, Wk)
    res = run_bass_kernel_spmd(nc, in_maps, core_ids=list(range(NCORES)))
    full = np.concatenate(
        [np.asarray(r["out"], dtype=np.float32) for r in res.results], axis=0
    )  # [Bk, Bq, C]
    return np.ascontiguousarray(full.transpose(1, 0, 2))  # [Bq, Bk, C]


# revision 27
# speedup vs baseline: 1.0114x; 1.0114x over previous
"""Trainium2 Bass kernel for bipartite cross-batch attention.

Reference computation (per full inputs):
  q  = LN(qx; gq,bq) @ Wq.T            -> [Bq, H, hd]
  k  = LN(kx; gk,bk) @ Wk.T            -> [Bk, Nk, H, hd]
  a  = softmax(q.k * hd^-0.5, axis=Nk) -> [Bq, Bk, H, Nk]
  w  = a.sum(H)                        -> [Bq, Bk, Nk]
  out= einsum('knc,qkn->qkc', kx, w)   -> [Bq, Bk, C]

Bq=128, Bk=128, Nk=256, C=1024, H=16, hd=64.

Distribution: shard Bk across the 8 cores (16 k-batches each); the softmax
axis Nk is fully core-local so there are no collectives.

Host-side prep: gq/gk fold into the projection weights; bk drops
(softmax-invariant); bq folds into a per-output-channel bias; hd^-0.5 and
the fp8 weight prescale (x32, exact power of two) fold into qT. The K-side
LayerNorm runs on the host while building the transposed copies of kx.

Precision: the K projection contracts C=1024 channels; the last NFP8
channels run as double-pumped fp8(e4m3) DoubleRow matmuls, the rest bf16.
Both weight halves are prescaled by 32 so fp8 weight magnitudes clear the
e4m3 subnormal range; qT carries the 1/32 so scores are exact.

Per-pair engine assignment (from measured op costs):
  - PE: projections, scores, 9 heads of the softmax head-sum as
    accumulating diag(1/d_h) matmuls into PSUM, w transposes, AV.
  - Act: kproj PSUM drains, exps, hsum drain, AV output drains.
  - DVE: softmax denominators (tensor_reduce), reciprocal, diag tile
    builds, 7 heads of the head-sum as scalar_tensor_tensor, merge add,
    wT copies.
  - Sync: all steady-state DMAs. GpSimd: startup DMAs only (its SBUF
    port is shared with the DVE, so giving it elementwise work slows
    the DVE down).
Pair p's den/chain run during p+1's projection stream; transposes/AV/
store land early in p+2.
"""

import numpy as np
import ml_dtypes

BF16 = ml_dtypes.bfloat16
E4M3 = ml_dtypes.float8_e4m3
H, C, HD = 16, 1024, 64
BQ, BK, NK = 128, 128, 256
NCORES = 8
BKL = BK // NCORES  # k-batches per core
PAIRS = BKL // 2
EPS = 1e-5

NFP8 = 512            # contract channels in fp8 (0, 256 or 512)
NBF = C - NFP8
NCHB = NBF // 128     # bf16 contract chunks
NCH8 = NFP8 // 256    # fp8 DoubleRow contract chunks
WSC = 32.0            # weight prescale (power of two; 1/WSC folded into qT)
HPE = 12              # heads summed on PE (diag matmuls); rest on DVE
DEN_A = 10            # heads of pair p's den reduced already in window p

_CACHE: dict = {}


def _build():
    from contextlib import ExitStack
    from concourse import bacc, tile, mybir

    f32 = mybir.dt.float32
    bf16 = mybir.dt.bfloat16
    f8e4 = mybir.dt.float8e4
    Alu = mybir.AluOpType
    Act = mybir.ActivationFunctionType
    DR = mybir.MatmulPerfMode.DoubleRow

    nc = bacc.Bacc("TRN2", target_bir_lowering=False, debug=False)

    kxtb_d = nc.dram_tensor(
        "kxtb", [PAIRS, 128, NCHB, 2 * NK], bf16, kind="ExternalInput").ap()
    if NCH8:
        kxt8_d = nc.dram_tensor(
            "kxt8", [PAIRS, 128, NCH8, 2, 2 * NK], f8e4, kind="ExternalInput").ap()
    kxn_d = nc.dram_tensor("kxn", [BKL, 128, 2, C], bf16, kind="ExternalInput").ap()
    qt_d = nc.dram_tensor("qt", [128, 8, 128], bf16, kind="ExternalInput").ap()
    wkb_d = nc.dram_tensor("wkb", [128, 8, NCHB, 128], bf16, kind="ExternalInput").ap()
    if NCH8:
        wk8_d = nc.dram_tensor(
            "wk8", [128, 8, NCH8, 2, 128], f8e4, kind="ExternalInput").ap()
    id_d = nc.dram_tensor("ident", [128, 128], bf16, kind="ExternalInput").ap()
    out_d = nc.dram_tensor("out", [BKL, BQ, C], bf16, kind="ExternalOutput").ap()

    with tile.TileContext(nc) as tc, ExitStack() as ctx:
        const = ctx.enter_context(tc.tile_pool(name="const", bufs=1))
        ktb_p = ctx.enter_context(tc.tile_pool(name="ktb", bufs=2))
        kt8_p = ctx.enter_context(tc.tile_pool(name="kt8", bufs=2))
        kn_p = ctx.enter_context(tc.tile_pool(name="kn", bufs=8))
        kj_p = ctx.enter_context(tc.tile_pool(name="kj", bufs=2))
        ex_p = ctx.enter_context(tc.tile_pool(name="ex", bufs=3))
        den_p = ctx.enter_context(tc.tile_pool(name="den", bufs=4))
        dg_p = ctx.enter_context(tc.tile_pool(name="dg", bufs=2))
        w_p = ctx.enter_context(tc.tile_pool(name="w", bufs=4))
        os_p = ctx.enter_context(tc.tile_pool(name="os", bufs=4))
        # PSUM: 8 banks: kp 2 + sc 3 + hs 1 + tp 1 + av 1
        pp_kp = ctx.enter_context(tc.tile_pool(name="pp_kp", bufs=2, space="PSUM"))
        pp_sc = ctx.enter_context(tc.tile_pool(name="pp_sc", bufs=3, space="PSUM"))
        pp_hs = ctx.enter_context(tc.tile_pool(name="pp_hs", bufs=1, space="PSUM"))
        pp_avt = ctx.enter_context(tc.tile_pool(name="pp_avt", bufs=1, space="PSUM"))

        # ---- constants ----
        wkb_t = const.tile([128, 8, NCHB, 128], bf16)
        if NCH8:
            wk8_t = const.tile([128, 8, NCH8, 2, 128], f8e4)
        id_t = const.tile([128, 128], bf16)
        qT = const.tile([128, 8, 128], bf16)

        ktb_tiles, kt8_tiles, kn_tiles = {}, {}, {}

        def emit_kt(bp, split=False):
            ktb = ktb_p.tile([128, NCHB, 2 * NK], bf16, tag="ktb")
            if split:
                for i in range(NCHB):
                    nc.sync.dma_start(ktb[:, i, :], kxtb_d[bp, :, i, :])
            else:
                nc.sync.dma_start(ktb[:], kxtb_d[bp])
            ktb_tiles[bp] = ktb
            if NCH8:
                kt8 = kt8_p.tile([128, NCH8, 2, 2 * NK], f8e4, tag="kt8")
                if split:
                    for d in range(NCH8):
                        nc.sync.dma_start(kt8[:, d, :, :], kxt8_d[bp, :, d, :, :])
                else:
                    nc.sync.dma_start(kt8[:], kxt8_d[bp])
                kt8_tiles[bp] = kt8

        def emit_kn(b, eng=None):
            kn_t = kn_p.tile([128, 2, C], bf16, tag="kn")
            (eng or nc.sync).dma_start(kn_t[:], kxn_d[b])
            kn_tiles[b] = kn_t

        # ---- startup DMAs (ordered for earliest proj(0,0) start) ----
        for i in range(NCHB):
            nc.scalar.dma_start(wkb_t[:, 0, i], wkb_d[:, 0, i])
        if NCH8:
            nc.scalar.dma_start(wk8_t[:, 0:2], wk8_d[:, 0:2])
        emit_kt(0, split=True)
        nc.scalar.dma_start(wkb_t[:, 1], wkb_d[:, 1])
        for j in range(2, 8):
            nc.gpsimd.dma_start(wkb_t[:, j], wkb_d[:, j])
        if NCH8:
            nc.scalar.dma_start(wk8_t[:, 2:8], wk8_d[:, 2:8])
        nc.gpsimd.dma_start(qT[:], qt_d[:])
        nc.gpsimd.dma_start(id_t[:], id_d[:])
        emit_kn(0, nc.gpsimd)
        emit_kn(1, nc.gpsimd)

        ex_tiles, wps_tiles, wsb_tiles, wacc_tiles = {}, {}, {}, {}
        w_tiles, iden_tiles, diag_tiles = {}, {}, {}

        den_tiles = {}

        def emit_den_a(bp):
            # DVE: dens for heads [0, DEN_A) of pair bp, in window bp
            extile = ex_tiles[bp]
            dens = den_p.tile([128, 32], f32, tag="dens")
            den_tiles[bp] = dens
            nc.vector.tensor_reduce(
                dens[:, 0:2 * DEN_A], extile[:, 0:DEN_A],
                mybir.AxisListType.X, Alu.add)

        def emit_den_b(bp):
            # DVE: remaining dens + reciprocal, in window bp+1
            extile = ex_tiles[bp]
            dens = den_tiles.pop(bp)
            nc.vector.tensor_reduce(
                dens[:, 2 * DEN_A:32], extile[:, DEN_A:16],
                mybir.AxisListType.X, Alu.add)
            idens = den_p.tile([128, 32], f32, tag="idens")
            nc.vector.reciprocal(idens[:], dens[:])
            iden_tiles[bp] = idens

        def emit_diag(bp, hs):
            # DVE: diag_h = ident * (1/d_h) as [128, 2, 128] per (h, t)
            idens = iden_tiles[bp]
            diag = diag_tiles.get(bp)
            if diag is None:
                diag = dg_p.tile([128, 16, 2, 128], bf16, tag="diag")
                diag_tiles[bp] = diag
            for h in hs:
                for t in range(2):
                    nc.vector.tensor_scalar(
                        diag[:, h, t, :], id_t[:],
                        idens[:, 2 * h + t:2 * h + t + 1], None, op0=Alu.mult)

        def emit_hsum_pe(bp, hs, start=False, stop=False):
            # PE: wps[q, t, n] += r_h[q] * e_h[q, t, n], accumulated in PSUM
            extile, diag = ex_tiles[bp], diag_tiles[bp]
            wps = wps_tiles.get(bp)
            if wps is None:
                wps = pp_hs.tile([128, 2, NK], f32, tag="hs")
                wps_tiles[bp] = wps
            for idx, h in enumerate(hs):
                for t in range(2):
                    nc.tensor.matmul(
                        wps[:, t, :], diag[:, h, t, :], extile[:, h, t, :],
                        start=(start and idx == 0 and t == 0),
                        stop=(stop and idx == len(hs) - 1 and t == 1),
                    )

        def emit_diag_act(bp, hs):
            # Act builds diag tiles (tail only; Act is idle there)
            idens = iden_tiles[bp]
            diag = diag_tiles.get(bp)
            if diag is None:
                diag = dg_p.tile([128, 16, 2, 128], bf16, tag="diag")
                diag_tiles[bp] = diag
            for h in hs:
                for t in range(2):
                    nc.scalar.activation(
                        diag[:, h, t, :], id_t[:], Act.Copy,
                        scale=idens[:, 2 * h + t:2 * h + t + 1])

        def emit_hsum_drain(bp):
            # tail only: all heads summed on PE; drain psum -> w tile
            wsb = w_p.tile([128, 2, NK], bf16, tag="wacc")
            nc.scalar.copy(wsb[:], wps_tiles.pop(bp)[:])
            diag_tiles.pop(bp)
            ex_tiles.pop(bp)
            iden_tiles.pop(bp)
            w_tiles[bp] = ("w", wsb)

        def emit_stt(bp, hs, seed=False):
            # DVE: wacc += e_h * (1/d_h); the first op seeds from the PE
            # half's PSUM accumulator directly (kills merge + drain)
            extile, idens = ex_tiles[bp], iden_tiles[bp]
            wacc = wacc_tiles.get(bp)
            if wacc is None:
                wacc = w_p.tile([128, 2, NK], bf16, tag="wacc")
                wacc_tiles[bp] = wacc
            for idx, h in enumerate(hs):
                for t in range(2):
                    in1 = (wps_tiles[bp][:, t, :] if (seed and idx == 0)
                           else wacc[:, t, :])
                    nc.vector.scalar_tensor_tensor(
                        wacc[:, t, :], extile[:, h, t, :],
                        idens[:, 2 * h + t:2 * h + t + 1], in1,
                        op0=Alu.mult, op1=Alu.add)

        def emit_finish(bp):
            # bookkeeping: wacc is the final w
            ex_tiles.pop(bp)
            iden_tiles.pop(bp)
            diag_tiles.pop(bp)
            wps_tiles.pop(bp)
            w_tiles[bp] = ("w", wacc_tiles.pop(bp))

        def emit_transpose_w(bp):
            # PE: 4 transposes into one psum bank; DVE: one [128,512] copy
            _, wfin = w_tiles.pop(bp)
            wT = w_p.tile([128, 2, 2, 128], bf16, tag="wT")
            wtp = pp_avt.tile([128, 512], bf16, tag="tp", bufs=1)
            for t in range(2):
                for u in range(2):
                    k = 2 * t + u
                    nc.tensor.transpose(
                        wtp[:, k * 128:(k + 1) * 128],
                        wfin[:, t, u * 128:(u + 1) * 128], id_t[:])
            nc.vector.tensor_copy(
                wT[:].rearrange("p t u q -> p (t u q)"), wtp[:])
            w_tiles[bp] = ("wT", wT)

        def emit_store_t(bp, t):
            # AV matmuls + Act drains + store for one batch of pair bp
            _, wT = w_tiles[bp]
            if t == 1:
                w_tiles.pop(bp)
            b = 2 * bp + t
            kn_t = kn_tiles.pop(b)
            out_sb = os_p.tile([BQ, C], bf16, tag="osb")
            for m in range(2):
                # m=1 borrows the transpose bank (free after j==1) so the
                # m=1 matmuls never wait on m=0's Act drain
                tag = "av" if m == 0 else "tp"
                avp = pp_avt.tile([BQ, 512], f32, tag=tag, bufs=1)
                for u in range(2):
                    nc.tensor.matmul(
                        avp[:], wT[:, t, u, :],
                        kn_t[:, u, m * 512:(m + 1) * 512],
                        start=(u == 0), stop=(u == 1),
                    )
                nc.scalar.copy(out_sb[:, m * 512:(m + 1) * 512], avp[:])
                nc.sync.dma_start(
                    out_d[b, :, m * 512:(m + 1) * 512],
                    out_sb[:, m * 512:(m + 1) * 512])

        HS_PE = list(range(HPE))           # heads on PE
        HS_DVE = list(range(HPE, 16))      # heads on DVE

        # ---- main paired loop ----
        for bp in range(PAIRS):
            ktb = ktb_tiles.pop(bp)
            kt8 = kt8_tiles.pop(bp) if NCH8 else None
            kjp = kj_p.tile([128, 8, 2 * NK], bf16, tag="kj")
            extile = ex_p.tile([128, 16, 2, NK], bf16, tag="ex")
            ex_tiles[bp] = extile

            def emit_kproj(j, ktb=ktb, kt8=kt8, kjp=kjp):
                kpp = pp_kp.tile([BQ, 2 * NK], f32, tag="kp")
                for i in range(NCHB):
                    nc.tensor.matmul(
                        kpp[:], wkb_t[:, j, i, :], ktb[:, i, :],
                        start=(i == 0), stop=(NCH8 == 0 and i == NCHB - 1),
                    )
                for d in range(NCH8):
                    nc.tensor.matmul(
                        kpp[:], wk8_t[:, j, d, :, :], kt8[:, d, :, :],
                        start=False, stop=(d == NCH8 - 1),
                        perf_mode=DR,
                    )
                nc.scalar.copy(kjp[:, j, :], kpp[:])

            def emit_score(h, kjp=kjp, extile=extile):
                j, off = h // 2, (h % 2) * 64
                scp = pp_sc.tile([BQ, 2 * NK], f32, tag="sc")
                nc.tensor.matmul(
                    scp[:], qT[off:off + 64, j, :], kjp[off:off + 64, j, :],
                    start=True, stop=True,
                )
                nc.scalar.activation(extile[:, h, :, :], scp[:], Act.Exp)

            for j in range(8):
                emit_kproj(j)
                if j == 0:
                    if bp + 1 < PAIRS:
                        emit_kt(bp + 1)
                        emit_kn(2 * bp + 2)
                        emit_kn(2 * bp + 3)
                    if bp >= 1:
                        emit_den_b(bp - 1)
                if j == 1:
                    if bp >= 2:
                        emit_transpose_w(bp - 2)
                    if bp >= 1:
                        emit_diag(bp - 1, HS_PE[0:4])
                if j == 2:
                    if bp >= 2:
                        emit_store_t(bp - 2, 0)
                    if bp >= 1:
                        emit_diag(bp - 1, HS_PE[4:8])
                        emit_hsum_pe(bp - 1, HS_PE[0:4], start=True)
                if j == 3 and bp >= 1:
                    emit_diag(bp - 1, HS_PE[8:HPE])
                    emit_hsum_pe(bp - 1, HS_PE[4:8])
                if j == 4:
                    if bp >= 2:
                        emit_store_t(bp - 2, 1)
                    if bp >= 1:
                        emit_hsum_pe(bp - 1, HS_PE[8:HPE], stop=True)
                if j == 5 and bp >= 1:
                    emit_stt(bp - 1, HS_DVE[0:2], seed=True)
                if j == 6:
                    if bp == PAIRS - 1:
                        # tail: let pair-6's chain finish before the den
                        # so the epilogue transposes unblock early
                        emit_stt(bp - 1, HS_DVE[2:4])
                        emit_den_a(bp)
                    else:
                        emit_den_a(bp)
                        if bp >= 1:
                            emit_stt(bp - 1, HS_DVE[2:4])
                if j == 7 and bp >= 1:
                    emit_finish(bp - 1)
                if j >= 1:
                    emit_score(2 * (j - 1))
                    emit_score(2 * (j - 1) + 1)
            for h in range(14, 16):
                emit_score(h)

        # ---- epilogue: flush pairs 6 and 7 ----
        bp = PAIRS - 1
        emit_transpose_w(bp - 1)
        emit_store_t(bp - 1, 0)
        emit_den_b(bp)
        emit_store_t(bp - 1, 1)
        # tail pair: all heads on PE; diag builds split DVE/Act
        # (Act copies cost ~480ns vs DVE ~254ns, so DVE takes more)
        emit_diag(bp, range(0, 11))
        emit_diag_act(bp, range(11, 16))
        emit_hsum_pe(bp, range(0, 11), start=True)
        emit_hsum_pe(bp, range(11, 16), stop=True)
        emit_hsum_drain(bp)
        emit_transpose_w(bp)
        emit_store_t(bp, 0)
        emit_store_t(bp, 1)

    nc.compile()
    return nc


def _prep(qx, kx, gq, bq, gk, bk, Wq, Wk):
    scale = HD ** -0.5
    qx_h = np.ascontiguousarray(qx[:, 0, :], dtype=np.float32)
    Wqp = (Wq * gq[None, :]).T.astype(np.float32) * scale  # [c, o]
    Wkp = (Wk * gk[None, :]).T.astype(np.float32)  # [c, o]
    # weight prescale x32 (power of 2): fp8 half clears e4m3 subnormals;
    # bf16 half matches so the PSUM accumulation is uniformly 32x; the
    # 1/32 folds into qT (scores exact).
    Wks = Wkp * WSC
    wkb_h = np.ascontiguousarray(
        Wks[:NBF].reshape(NCHB, 128, 8, 128).transpose(1, 2, 0, 3)
    ).astype(BF16)  # [p, j, i, o]
    if NCH8:
        wk8_h = np.ascontiguousarray(
            Wks[NBF:].reshape(NCH8, 128, 2, 8, 128).transpose(1, 3, 0, 2, 4)
        ).astype(E4M3)  # [p, j, d, s, o]

    qm = qx_h.mean(-1, keepdims=True)
    qv = qx_h.var(-1, keepdims=True)
    lnq = ((qx_h - qm) / np.sqrt(qv + EPS)).astype(BF16).astype(np.float32)
    q = lnq @ Wqp.astype(BF16).astype(np.float32)
    q += scale * (bq[None, :] @ Wq.T)
    qt_h = np.ascontiguousarray(
        (q / WSC).T.reshape(8, 128, 128).transpose(1, 0, 2)).astype(BF16)
    id_h = np.eye(128, dtype=np.float32).astype(BF16)

    shared = dict(qt=qt_h, wkb=wkb_h, ident=id_h)
    if NCH8:
        shared["wk8"] = wk8_h
    in_maps = []
    for i in range(NCORES):
        kxl = np.asarray(kx[i * BKL:(i + 1) * BKL], dtype=np.float32)
        m = kxl.mean(axis=-1, keepdims=True)
        v = kxl.var(axis=-1, keepdims=True)
        kln = (kxl - m) / np.sqrt(v + EPS)  # [b, n, c]
        klt = (
            kln.reshape(PAIRS, 2, NK, C)
            .transpose(0, 3, 1, 2)  # [bp, c, t, n]
            .reshape(PAIRS, C, 2 * NK)
        )
        kxtb_h = np.ascontiguousarray(
            klt[:, :NBF].reshape(PAIRS, NCHB, 128, 2 * NK).transpose(0, 2, 1, 3)
        ).astype(BF16)  # [bp, p, i, tn]
        d = dict(kxtb=kxtb_h, **shared)
        if NCH8:
            d["kxt8"] = np.ascontiguousarray(
                klt[:, NBF:].reshape(PAIRS, NCH8, 128, 2, 2 * NK)
                .transpose(0, 2, 1, 3, 4)
            ).astype(E4M3)  # [bp, p, d, s, tn]
        d["kxn"] = np.ascontiguousarray(
            kxl.reshape(BKL, 2, 128, C).transpose(0, 2, 1, 3)
        ).astype(BF16)
        in_maps.append(d)
    return in_maps


def kernel(qx, kx, gq, bq, gk, bk, Wq, Wk):
    from concourse.bass_utils import run_bass_kernel_spmd

    qx, kx, gq, bq, gk, bk, Wq, Wk = (
        np.asarray(a, dtype=np.float32)
        for a in (qx, kx, gq, bq, gk, bk, Wq, Wk)
    )
    if "nc" not in _CACHE:
        _CACHE["nc"] = _build()
    nc = _CACHE["nc"]
    in_maps = _prep(qx, kx, gq, bq, gk, bk, Wq, Wk)
    res = run_bass_kernel_spmd(nc, in_maps, core_ids=list(range(NCORES)))
    full = np.concatenate(
        [np.asarray(r["out"], dtype=np.float32) for r in res.results], axis=0
    )  # [Bk, Bq, C]
    return np.ascontiguousarray(full.transpose(1, 0, 2))  # [Bq, Bk, C]
